# revision 30
# baseline (speedup 1.0000x reference)
"""Multi-head self-attention (B=2, T=2048, D=1024, 16 heads) on 8 TRN2 cores.

Sharding: core c = (b, g) with b = c // 4 (batch), g = c % 4 (head group of 4).
Each core computes q/k/v projections for its 4 heads, causal softmax
attention, and a partial output projection (its 256 columns of the
concat-head dim against Wo). Host sums the 4 partials per batch and adds bo.

The schedule is one flat stream of attention score-chunks (block = (J query
block of 512, hp head pair); chunk = 128 keys) with the scores matmuls
emitted two chunks ahead of their AV consumers, so the scalar engine's Exp
stream — the serial resource of the attention middle — runs back-to-back.
Everything else is demand-driven around that stream:

  - A dozen junk matmuls at t=0 warm the PE's HAM clock gate to 2.4 GHz
    while the first input DMAs (spread over 4 engine queues) land.
  - Projection work (qk per token-quarter, v per key-chunk) is emitted
    lazily: the minimum needed before the first attention block, the rest
    as PE filler popped between chunks / at block starts, with hard
    dependencies ensured at the block that needs them.
  - Softmax: AV accumulates [v|1].T @ exp into at [65,512] PSUM (row 64 =
    denominator via the ones column of v, which also applies the key
    padding mask); at block end one DVE cast stages at into SBUF f32 and
    frees the PSUM bank fast; the reciprocal/broadcast/normalize-multiply
    run lazily off the critical path (DVE + gpsimd), writing bf16 attT.
  - ACT does only Exp (one LUT load) plus the qk bias-add copies
    (Identity + per-partition bias); v masking is a DVE tensor_scalar;
    O-projection PSUM->SBUF casts are DVE; O is stored bf16.
  - PSUM: scores 2x[128,1024] (4 banks) + at 2x[128,512] (2 banks) +
    shared aux tag (2 banks) for warmup/qk/v/O-projection accumulators.

Scores chunks [tk=128, tq<=512] = kT.T @ qT with the head pair row-packed
via tile_position (0,0)/(64,0); diagonal chunks column-restricted to the
causal region and tri-masked on their first 128 columns (DVE multiply).
"""

import ml_dtypes
import numpy as np

import concourse.bass as bass
import concourse.tile as tile
from concourse import bacc, mybir
from concourse import bass_utils
from contextlib import ExitStack

F32 = mybir.dt.float32
BF16 = mybir.dt.bfloat16
AF = mybir.ActivationFunctionType
OP = mybir.AluOpType

B, T, D = 2, 2048, 1024
NH, DH = 16, 64
HPC = 4              # heads per core
GD = HPC * DH        # 256, group dim
GV = HPC * (DH + 1)  # 260, packed v width (64 v dims + ones col per head)
NKD = D // 128       # 8 K-chunks for projections
NT = T // 128        # 16 token chunks
NJ = T // 512        # 4 query blocks

BLOCKS = [(0, 0), (0, 1), (1, 0), (1, 1), (2, 0), (2, 1), (3, 0), (3, 1)]

_NC_CACHE = {}


def build():
    if "nc" in _NC_CACHE:
        return _NC_CACHE["nc"]
    nc = bacc.Bacc("TRN2", target_bir_lowering=False, debug=False, num_devices=8)

    HT = nc.dram_tensor("HT", [D, T], BF16, kind="ExternalInput").ap()
    WqP = nc.dram_tensor("WqP", [128, NKD * GD], BF16, kind="ExternalInput").ap()
    WkP = nc.dram_tensor("WkP", [128, NKD * GD], BF16, kind="ExternalInput").ap()
    WvP = nc.dram_tensor("WvP", [128, NKD * GV], BF16, kind="ExternalInput").ap()
    WoP = nc.dram_tensor("WoP", [128, 2 * D], BF16, kind="ExternalInput").ap()
    bqk = nc.dram_tensor("bqk", [128, 4], F32, kind="ExternalInput").ap()
    bvP = nc.dram_tensor("bvP", [1, GV], BF16, kind="ExternalInput").ap()
    kpm = nc.dram_tensor("kpm", [128, NT], F32, kind="ExternalInput").ap()
    O = nc.dram_tensor("O", [T, D], BF16, kind="ExternalOutput").ap()

    with tile.TileContext(nc) as tc, ExitStack() as octx:
        cpool = octx.enter_context(tc.tile_pool(name="const", bufs=1))
        keep = octx.enter_context(tc.tile_pool(name="keep", bufs=1))
        work = octx.enter_context(tc.tile_pool(name="work", bufs=1))
        psc = octx.enter_context(tc.tile_pool(name="psc", bufs=1, space="PSUM"))
        pav = octx.enter_context(tc.tile_pool(name="pav", bufs=1, space="PSUM"))
        paux = octx.enter_context(tc.tile_pool(name="paux", bufs=1, space="PSUM"))

        # ---- PE warm-up: junk matmuls at t=0 ramp the HAM clock gate while
        # the first input DMAs land. Nothing reads the results.
        junk = cpool.tile([128, 512], BF16, name="junk", tag="junk")
        nc.vector.memset(junk[:], 0.0)
        for _ in range(34):
            wp = paux.tile([128, 512], F32, name="wp", tag="aux", bufs=2)
            nc.tensor.matmul(wp[:, 0:256], junk[:, 0:128], junk[:, 0:256],
                             start=True, stop=True)

        # ---- constants (small DMAs first on the gpsimd queue) ----
        ones_r = cpool.tile([1, 128], BF16, name="ones_r", tag="ones_r")
        nc.vector.memset(ones_r[:], 1.0)
        bqk_sb = cpool.tile([128, 4], F32, name="bqk_sb", tag="bqk_sb")
        nc.gpsimd.dma_start(bqk_sb[:], bqk[:])
        bv_r = cpool.tile([1, GV], BF16, name="bv_r", tag="bv_r")
        nc.gpsimd.dma_start(bv_r[:], bvP[:])
        kpm_sb = cpool.tile([128, NT], F32, name="kpm_sb", tag="kpm_sb")
        nc.gpsimd.dma_start(kpm_sb[:], kpm[:])

        # ---- long-lived tiles ----
        qT = [keep.tile([128, T], BF16, name=f"qT{m}", tag=f"qT{m}") for m in range(2)]
        kT = [keep.tile([128, T], BF16, name=f"kT{m}", tag=f"kT{m}") for m in range(2)]
        vt = [keep.tile([128, GV], BF16, name=f"vt{t}", tag=f"vt{t}") for t in range(NT)]
        attT = [keep.tile([128, T], BF16, name=f"attT{m}", tag=f"attT{m}") for m in range(2)]
        ht_r = [keep.tile([128, T], BF16, name=f"ht{k}", tag=f"ht{k}") for k in range(NKD)]
        wq_r = keep.tile([128, NKD * GD], BF16, name="wq_r", tag="wq_r")
        wk_r = keep.tile([128, NKD * GD], BF16, name="wk_r", tag="wk_r")
        wv_r = keep.tile([128, NKD * GV], BF16, name="wv_r", tag="wv_r")
        wo_r = keep.tile([128, 2 * D], BF16, name="wo_r", tag="wo_r")

        # ---- input DMA: sync + gpsimd only. The scalar (ACT) queue must
        # stay clean — DMA descriptor preps and ring-full waits on it would
        # stall the exp stream behind them.
        nc.sync.dma_start(wq_r[:], WqP[:])
        for k in range(0, NKD, 2):
            nc.sync.dma_start(ht_r[k][:, 0:1024], HT[k * 128:(k + 1) * 128, 0:1024])
        for k in range(1, NKD, 2):
            nc.gpsimd.dma_start(ht_r[k][:, 0:1024], HT[k * 128:(k + 1) * 128, 0:1024])
        nc.sync.dma_start(wk_r[:], WkP[:])
        nc.gpsimd.dma_start(wv_r[:], WvP[:])
        for k in range(NKD):
            nc.sync.dma_start(ht_r[k][:, 1024:2048], HT[k * 128:(k + 1) * 128, 1024:2048])
        nc.gpsimd.dma_start(wo_r[:], WoP[:])

        # lower-tri mask [128,128]: keep where free >= partition
        tri = cpool.tile([128, 128], BF16, name="tri", tag="tri")
        nc.gpsimd.memset(tri[:], 1.0)
        nc.gpsimd.affine_select(
            out=tri[:], in_=tri[:], compare_op=OP.is_ge, fill=0.0,
            base=0, pattern=[[1, 128]], channel_multiplier=-1,
        )

        # ---- emission-time clock model (us) for filler pacing ----
        # PE at 2.4 GHz, ACT at 1.2 GHz, DVE at 0.96 GHz; coarse per-op
        # costs including access penalties. Used only to decide how much
        # projection filler to pack between attention chunks.
        clk = {"pe": 0.0, "act": 0.0, "dve": 0.0}

        def mm_cost(n_cols):
            return n_cols / 2400.0

        def dve_cost(n_free):
            return (n_free + 120) / 960.0

        # ---- units, split into single-matmul steps for fine interleaving ----
        emitted = set()
        ps_of = {}

        def qk_steps(di, m, n):
            # 8 accumulation matmuls + one DVE bias-add copy-out
            key = ("qk", di, m, n)
            if key in emitted:
                return []
            emitted.add(key)
            dest, w_r = ((qT, wq_r), (kT, wk_r))[di]

            def mk_mm(k):
                def f():
                    if k == 0:
                        ps_of[key] = paux.tile(
                            [128, 512], F32, name="ps", tag="aux", bufs=2
                        )
                    nc.tensor.matmul(
                        ps_of[key][:],
                        w_r[:, k * GD + m * 128: k * GD + m * 128 + 128],
                        ht_r[k][:, n * 512:(n + 1) * 512],
                        start=(k == 0), stop=(k == NKD - 1),
                    )
                    clk["pe"] += mm_cost(512)
                return f

            def fin():
                nc.vector.tensor_scalar_add(
                    dest[m][:, n * 512:(n + 1) * 512], ps_of.pop(key)[:],
                    bqk_sb[:, 2 * di + m: 2 * di + m + 1],
                )
                clk["dve"] = max(clk["dve"], clk["pe"]) + dve_cost(512)

            return [mk_mm(k) for k in range(NKD)] + [fin]

        def v_steps(t):
            key = ("v", t)
            if key in emitted:
                return []
            emitted.add(key)

            def mk_mm(k):
                def f():
                    if k == 0:
                        ps_of[key] = paux.tile(
                            [128, 512], F32, name="vp", tag="aux", bufs=2
                        )
                    nc.tensor.matmul(
                        ps_of[key][:, 0:GV],
                        ht_r[k][:, t * 128:(t + 1) * 128],
                        wv_r[:, k * GV:(k + 1) * GV],
                        start=(k == 0), stop=False,
                    )
                    clk["pe"] += mm_cost(GV)
                return f

            def bias_mm():
                nc.tensor.matmul(
                    ps_of[key][:, 0:GV], ones_r[:], bv_r[:], start=False, stop=True
                )
                clk["pe"] += mm_cost(GV)

            def fin():
                nc.vector.tensor_scalar_mul(
                    vt[t][:], ps_of.pop(key)[:, 0:GV], kpm_sb[:, t:t + 1]
                )
                clk["dve"] = max(clk["dve"], clk["pe"]) + dve_cost(GV)

            return [mk_mm(k) for k in range(NKD)] + [bias_mm, fin]

        def c_steps(t):
            ot_box = []
            steps = []
            for n in range(2):
                def mk(n=n):
                    def f():
                        if not ot_box:
                            ot_box.append(
                                work.tile([128, D], BF16, name="ot", tag="ot", bufs=3)
                            )
                        op = paux.tile([128, 512], F32, name="op", tag="aux", bufs=2)
                        for hp in range(2):
                            nc.tensor.matmul(
                                op[:],
                                attT[hp][:, t * 128:(t + 1) * 128],
                                wo_r[:, hp * D + n * 512: hp * D + (n + 1) * 512],
                                start=(hp == 0), stop=(hp == 1),
                            )
                        clk["pe"] += 2 * mm_cost(512)
                        if t >= 12 and n == 1:
                            # tail: exp stream is over, use the idle ACT
                            nc.scalar.copy(ot_box[0][:, n * 512:(n + 1) * 512], op[:])
                        else:
                            nc.vector.tensor_copy(
                                ot_box[0][:, n * 512:(n + 1) * 512], op[:]
                            )
                            clk["dve"] = max(clk["dve"], clk["pe"]) + dve_cost(512)
                    return f
                steps.append(mk())

            def fin():
                nc.sync.dma_start(O[t * 128:(t + 1) * 128, :], ot_box[0][:])

            steps.append(fin)
            return steps

        from collections import deque

        # filler machinery: units are deques of single-op steps; pop_filler
        # advances one step of the front unit; ensure() flushes a unit's
        # remaining steps immediately (hard dependency reached its deadline).
        unit_q = {}
        filler_order = deque()
        open_unit = [None]  # partially-drained filler unit (holds an aux slot)

        def queue_unit(key, steps):
            if steps:
                unit_q[key] = deque(steps)
                filler_order.append(key)

        def _flush(key):
            q = unit_q.pop(key, None)
            if open_unit[0] == key:
                open_unit[0] = None
            while q:
                q.popleft()()

        def ensure(key, steps_fn):
            # The shared-aux PSUM ring has 2 slots: a mid-flight filler unit
            # holds one, an inline ensure takes the other. Flush the open
            # filler first so consecutive ensures can't rotate onto it.
            if open_unit[0] is not None and open_unit[0] != key:
                _flush(open_unit[0])
            if key in unit_q:
                _flush(key)
            else:
                for s in steps_fn():
                    s()

        def pop_filler():
            while filler_order:
                key = filler_order[0]
                q = unit_q.get(key)
                if not q:
                    filler_order.popleft()
                    unit_q.pop(key, None)
                    if open_unit[0] == key:
                        open_unit[0] = None
                    continue
                q.popleft()()
                if not q:
                    filler_order.popleft()
                    unit_q.pop(key, None)
                    if open_unit[0] == key:
                        open_unit[0] = None
                else:
                    open_unit[0] = key
                return True
            return False

        def fillers_left():
            return bool(filler_order)

        def steps_until(keys):
            # steps remaining in the queue prefix that covers all of `keys`
            want = set(k for k in keys if k in unit_q)
            if not want:
                return 0
            tot = 0
            for k in filler_order:
                tot += len(unit_q.get(k, ()))
                want.discard(k)
                if not want:
                    break
            return tot

        # queue order: per J group, deps-first — (J,0)'s scores pair and
        # first diag v, then (J,1)'s scores pair, then the tail diag v's
        queue_unit(("qk", 0, 1, 0), qk_steps(0, 1, 0))
        queue_unit(("qk", 1, 1, 0), qk_steps(1, 1, 0))
        for J in range(1, NJ):
            queue_unit(("qk", 0, 0, J), qk_steps(0, 0, J))
            queue_unit(("qk", 1, 0, J), qk_steps(1, 0, J))
            queue_unit(("v", 4 * J), v_steps(4 * J))
            queue_unit(("qk", 0, 1, J), qk_steps(0, 1, J))
            queue_unit(("qk", 1, 1, J), qk_steps(1, 1, J))
            for t in range(4 * J + 1, 4 * J + 4):
                queue_unit(("v", t), v_steps(t))

        # ---- attention chunk stream ----
        chunks = []
        for J, hp in BLOCKS:
            kcs = [4 * J] + list(range(4 * J)) + [4 * J + i for i in range(1, 4)]
            for ti, kc in enumerate(kcs):
                chunks.append((J, hp, kc, ti, len(kcs)))

        ex_of = {}
        at_of = {}
        at_free = [0.0]  # est time the previous block's at pair frees

        def sc_exp(idx):
            J, hp, kc, ti, _n = chunks[idx]
            if ti == 0:
                ensure(("qk", 0, hp, J), lambda: qk_steps(0, hp, J))
                ensure(("qk", 1, hp, J), lambda: qk_steps(1, hp, J))
            off = max(0, 128 * (kc - 4 * J))
            w = 512 - off
            sc = psc.tile([128, 1024], F32, name="sc", tag="sc", bufs=2)
            for hh in range(2):
                nc.tensor.matmul(
                    sc[:, hh * 512:hh * 512 + w],
                    kT[hp][hh * 64:(hh + 1) * 64, kc * 128:(kc + 1) * 128],
                    qT[hp][hh * 64:(hh + 1) * 64, J * 512 + off:(J + 1) * 512],
                    start=True, stop=True,
                    tile_position=(hh * 64, 0),
                )
            clk["pe"] += mm_cost(w)
            ex = work.tile([128, 1024], BF16, name="ex", tag="ex", bufs=8)
            nc.scalar.activation(
                ex[:].rearrange("p (h c) -> p h c", c=512)[:, :, 0:w],
                sc[:].rearrange("p (h c) -> p h c", c=512)[:, :, 0:w],
                AF.Exp, scale=0.125,
            )
            clk["act"] = max(clk["act"], clk["pe"] + 0.1) + (2 * w + 420) / 1200.0
            ex_ready = clk["act"]
            if off or kc == 4 * J:
                for hh in range(2):
                    nc.vector.tensor_tensor(
                        ex[:, hh * 512:hh * 512 + 128],
                        ex[:, hh * 512:hh * 512 + 128],
                        tri[:],
                        op=OP.mult,
                    )
                clk["dve"] = max(clk["dve"], clk["act"]) + 2 * dve_cost(128)
                ex_ready = clk["dve"]
            ex_of[idx] = (ex, ex_ready)

        def issue_av(idx):
            J, hp, kc, ti, n = chunks[idx]
            ex, ex_ready = ex_of.pop(idx)
            ensure(("v", kc), lambda: v_steps(kc))
            if ti == 0:
                at_of[(J, hp)] = [
                    pav.tile([128, 512], F32, name=f"at{hh}", tag="av", bufs=2)
                    for hh in range(2)
                ]
            at = at_of[(J, hp)]
            off = max(0, 128 * (kc - 4 * J))
            w = 512 - off
            for hh in range(2):
                h = 2 * hp + hh
                nc.tensor.matmul(
                    at[hh][0:65, off:512],
                    vt[kc][:, h * 65:(h + 1) * 65],
                    ex[:, hh * 512:hh * 512 + w],
                    start=(ti == 0), stop=(ti == n - 1),
                )
            dep = max(ex_ready, at_free[0] if ti == 0 else 0.0)
            clk["pe"] = max(clk["pe"], dep) + 2 * mm_cost(w)

        def post_block(J, hp):
            # softmax normalize: 1/denom (row 64), broadcast, multiply the
            # PSUM at rows into bf16 attT.
            at = at_of.pop((J, hp))
            sdens = []
            for hh in range(2):
                sden = work.tile([1, 512], F32, name="sden", tag="sden", bufs=4)
                nc.vector.tensor_copy(sden[:], at[hh][64:65, :])
                sdens.append(sden)
            clk["dve"] = max(clk["dve"], clk["pe"]) + 2 * dve_cost(512)
            for hh in range(2):
                rden = work.tile([1, 512], F32, name="rden", tag="rden", bufs=4)
                nc.vector.reciprocal_approx_fast(rden[:], sdens[hh][:])
                rb = work.tile([64, 512], F32, name="rb", tag="rb", bufs=4)
                nc.gpsimd.partition_broadcast(rb[:], rden[:])
                nc.vector.tensor_tensor(
                    attT[hp][hh * 64:(hh + 1) * 64, J * 512:(J + 1) * 512],
                    at[hh][0:64, :],
                    rb[:],
                    op=OP.mult,
                )
            clk["dve"] += 2 * (dve_cost(512) + dve_cost(512)) + 1.2
            at_free[0] = clk["dve"]
            if hp == 1:
                for t in range(4 * J, 4 * J + 4):
                    queue_unit(("c", t), c_steps(t))

        def block_deps(bi):
            # unit keys a block needs before its scores / first AV; later
            # vt are streamed per-chunk by issue_av's ensure
            if bi >= len(BLOCKS):
                return []
            J, hp = BLOCKS[bi]
            return [("qk", 0, hp, J), ("qk", 1, hp, J), ("v", 4 * J)]

        # ---- prologue: minimum before the first exp, then the stream ----
        ensure(("qk", 0, 0, 0), lambda: qk_steps(0, 0, 0))
        ensure(("qk", 1, 0, 0), lambda: qk_steps(1, 0, 0))
        sc_exp(0)
        sc_exp(1)
        bi = 0
        for idx, (J, hp, kc, ti, n) in enumerate(chunks):
            # deadline prefetch (next block's projection deps, spread over
            # this block) + fixed ~0.55us pacing, hard-capped per chunk so
            # the one-chunk exp pipeline buffer never drains (ACT starves
            # after >1 chunk-time of consecutive PE filler).
            rem = steps_until(block_deps(bi + 1))
            left = (n - ti) - 2
            rate = rem if left <= 0 else -(-rem // left)
            pe0 = clk["pe"]
            budget = 1.3 if ti == 0 else 0.9
            popped = 0
            while fillers_left() and clk["pe"] - pe0 < budget:
                if popped >= rate and clk["pe"] - pe0 >= 0.55:
                    break
                pop_filler()
                popped += 1
            if idx + 2 < len(chunks):
                sc_exp(idx + 2)
            issue_av(idx)
            if ti == n - 1:
                post_block(J, hp)
                bi += 1

        # ---- tail: remaining output chunks ----
        while pop_filler():
            pass

    nc.compile()
    _NC_CACHE["nc"] = nc
    return nc


def _prep_core_inputs(H, key_padding_mask, Wq, bq, Wk, bk, Wv, bv, Wo, bo):
    keep = 1.0 - np.asarray(key_padding_mask, dtype=np.float32)  # [B, T]
    bf = ml_dtypes.bfloat16
    in_maps = []
    for c in range(8):
        b, g = divmod(c, 4)
        sl = slice(g * GD, (g + 1) * GD)
        WqT = np.ascontiguousarray(Wq[sl].T)  # [D, GD]
        WkT = np.ascontiguousarray(Wk[sl].T)
        WvT = Wv[sl].T  # [D, GD]
        WvS = np.zeros((D, GV), dtype=np.float32)
        bvS = np.zeros((1, GV), dtype=np.float32)
        for h in range(HPC):
            WvS[:, h * 65:h * 65 + 64] = WvT[:, h * 64:(h + 1) * 64]
            bvS[0, h * 65:h * 65 + 64] = bv[sl][h * 64:(h + 1) * 64]
            bvS[0, h * 65 + 64] = 1.0
        # pack weight k-chunks side by side: [128, NKD*width]
        WqPk = WqT.reshape(NKD, 128, GD).transpose(1, 0, 2).reshape(128, NKD * GD)
        WkPk = WkT.reshape(NKD, 128, GD).transpose(1, 0, 2).reshape(128, NKD * GD)
        WvPk = WvS.reshape(NKD, 128, GV).transpose(1, 0, 2).reshape(128, NKD * GV)
        WoS = np.ascontiguousarray(Wo[:, sl].T)  # [GD, D]
        WoPk = WoS.reshape(2, 128, D).transpose(1, 0, 2).reshape(128, 2 * D)
        bqk_m = np.stack(
            [bq[sl][0:128], bq[sl][128:256], bk[sl][0:128], bk[sl][128:256]], axis=1
        )  # [128, 4]
        in_maps.append({
            "HT": np.ascontiguousarray(H[b].T).astype(bf),
            "WqP": np.ascontiguousarray(WqPk).astype(bf),
            "WkP": np.ascontiguousarray(WkPk).astype(bf),
            "WvP": np.ascontiguousarray(WvPk).astype(bf),
            "WoP": np.ascontiguousarray(WoPk).astype(bf),
            "bqk": np.ascontiguousarray(bqk_m.astype(np.float32)),
            "bvP": bvS.astype(bf),
            "kpm": np.ascontiguousarray(keep[b].reshape(NT, 128).T),
        })
    return in_maps


def kernel(H, key_padding_mask, Wq, bq, Wk, bk, Wv, bv, Wo, bo, _run_kwargs=None):
    H = np.asarray(H, dtype=np.float32)
    Wq = np.asarray(Wq, dtype=np.float32)
    Wk = np.asarray(Wk, dtype=np.float32)
    Wv = np.asarray(Wv, dtype=np.float32)
    Wo = np.asarray(Wo, dtype=np.float32)
    bq = np.asarray(bq, dtype=np.float32)
    bk = np.asarray(bk, dtype=np.float32)
    bv = np.asarray(bv, dtype=np.float32)
    bo = np.asarray(bo, dtype=np.float32)

    nc = build()
    in_maps = _prep_core_inputs(H, key_padding_mask, Wq, bq, Wk, bk, Wv, bv, Wo, bo)
    res = bass_utils.run_bass_kernel_spmd(
        nc, in_maps, core_ids=list(range(8)), **(_run_kwargs or {})
    )
    out = np.zeros((B, T, D), dtype=np.float32)
    for c in range(8):
        out[c // 4] += np.asarray(res.results[c]["O"], dtype=np.float32)
    out += bo
    if _run_kwargs:
        kernel.last_result = res
    return out


# revision 36
# speedup vs baseline: 1.0374x; 1.0374x over previous
"""Multi-head self-attention (B=2, T=2048, D=1024, 16 heads) on 8 TRN2 cores.

Sharding: core c = (b, g) with b = c // 4 (batch), g = c % 4 (head group of 4).
Each core computes q/k/v projections for its 4 heads, causal softmax
attention, and a partial output projection (its 256 columns of the
concat-head dim against Wo). Host sums the 4 partials per batch and adds bo.

The schedule is one flat stream of attention score-chunks (block = (J query
block of 512, hp head pair); chunk = 128 keys) with the scores matmuls
emitted two chunks ahead of their AV consumers, so the scalar engine's Exp
stream — the serial resource of the attention middle — runs back-to-back.
Everything else is demand-driven around that stream:

  - A dozen junk matmuls at t=0 warm the PE's HAM clock gate to 2.4 GHz
    while the first input DMAs (spread over 4 engine queues) land.
  - Projection work (qk per token-quarter, v per key-chunk) is emitted
    lazily: the minimum needed before the first attention block, the rest
    as PE filler popped between chunks / at block starts, with hard
    dependencies ensured at the block that needs them.
  - Softmax: AV accumulates [v|1].T @ exp into at [65,512] PSUM (row 64 =
    denominator via the ones column of v, which also applies the key
    padding mask); at block end one DVE cast stages at into SBUF f32 and
    frees the PSUM bank fast; the reciprocal/broadcast/normalize-multiply
    run lazily off the critical path (DVE + gpsimd), writing bf16 attT.
  - ACT does only Exp (one LUT load) plus the qk bias-add copies
    (Identity + per-partition bias); v masking is a DVE tensor_scalar;
    O-projection PSUM->SBUF casts are DVE; O is stored bf16.
  - PSUM: scores 2x[128,1024] (4 banks) + at 2x[128,512] (2 banks) +
    shared aux tag (2 banks) for warmup/qk/v/O-projection accumulators.

Scores chunks [tk=128, tq<=512] = kT.T @ qT with the head pair row-packed
via tile_position (0,0)/(64,0); diagonal chunks column-restricted to the
causal region and tri-masked on their first 128 columns (DVE multiply).
"""

import ml_dtypes
import numpy as np

import concourse.bass as bass
import concourse.tile as tile
from concourse import bacc, mybir
from concourse import bass_utils
from contextlib import ExitStack

F32 = mybir.dt.float32
BF16 = mybir.dt.bfloat16
AF = mybir.ActivationFunctionType
OP = mybir.AluOpType

B, T, D = 2, 2048, 1024
NH, DH = 16, 64
HPC = 4              # heads per core
GD = HPC * DH        # 256, group dim
GV = HPC * (DH + 1)  # 260, packed v width (64 v dims + ones col per head)
NKD = D // 128       # 8 K-chunks for projections
NT = T // 128        # 16 token chunks
NJ = T // 512        # 4 query blocks

BLOCKS = [(0, 0), (0, 1), (1, 0), (1, 1), (2, 0), (2, 1), (3, 0), (3, 1)]

_NC_CACHE = {}


def build():
    if "nc" in _NC_CACHE:
        return _NC_CACHE["nc"]
    nc = bacc.Bacc("TRN2", target_bir_lowering=False, debug=False, num_devices=8)

    HT = nc.dram_tensor("HT", [D, T], BF16, kind="ExternalInput").ap()
    WqP = nc.dram_tensor("WqP", [128, NKD * GD], BF16, kind="ExternalInput").ap()
    WkP = nc.dram_tensor("WkP", [128, NKD * GD], BF16, kind="ExternalInput").ap()
    WvP = nc.dram_tensor("WvP", [128, NKD * GV], BF16, kind="ExternalInput").ap()
    WoP = nc.dram_tensor("WoP", [128, 2 * D], BF16, kind="ExternalInput").ap()
    bqk = nc.dram_tensor("bqk", [128, 4], F32, kind="ExternalInput").ap()
    bvP = nc.dram_tensor("bvP", [1, GV], BF16, kind="ExternalInput").ap()
    kpm = nc.dram_tensor("kpm", [128, NT], F32, kind="ExternalInput").ap()
    O = nc.dram_tensor("O", [T, D], BF16, kind="ExternalOutput").ap()

    with tile.TileContext(nc) as tc, ExitStack() as octx:
        cpool = octx.enter_context(tc.tile_pool(name="const", bufs=1))
        keep = octx.enter_context(tc.tile_pool(name="keep", bufs=1))
        work = octx.enter_context(tc.tile_pool(name="work", bufs=1))
        psc = octx.enter_context(tc.tile_pool(name="psc", bufs=1, space="PSUM"))
        pav = octx.enter_context(tc.tile_pool(name="pav", bufs=1, space="PSUM"))
        paux = octx.enter_context(tc.tile_pool(name="paux", bufs=1, space="PSUM"))

        # ---- PE warm-up: junk matmuls at t=0 ramp the HAM clock gate while
        # the first input DMAs land. Nothing reads the results.
        junk = cpool.tile([128, 512], BF16, name="junk", tag="junk")
        nc.vector.memset(junk[:], 0.0)
        for _ in range(26):
            wp = paux.tile([128, 512], F32, name="wp", tag="aux", bufs=2)
            nc.tensor.matmul(wp[:, 0:256], junk[:, 0:128], junk[:, 0:256],
                             start=True, stop=True)

        # ---- constants (small DMAs first on the gpsimd queue) ----
        ones_r = cpool.tile([1, 128], BF16, name="ones_r", tag="ones_r")
        nc.vector.memset(ones_r[:], 1.0)
        bqk_sb = cpool.tile([128, 4], F32, name="bqk_sb", tag="bqk_sb")
        nc.gpsimd.dma_start(bqk_sb[:], bqk[:])
        bv_r = cpool.tile([1, GV], BF16, name="bv_r", tag="bv_r")
        nc.gpsimd.dma_start(bv_r[:], bvP[:])
        kpm_sb = cpool.tile([128, NT], F32, name="kpm_sb", tag="kpm_sb")
        nc.gpsimd.dma_start(kpm_sb[:], kpm[:])

        # ---- long-lived tiles ----
        qT = [keep.tile([128, T], BF16, name=f"qT{m}", tag=f"qT{m}") for m in range(2)]
        kT = [keep.tile([128, T], BF16, name=f"kT{m}", tag=f"kT{m}") for m in range(2)]
        vt = [keep.tile([128, GV], BF16, name=f"vt{t}", tag=f"vt{t}") for t in range(NT)]
        attT = [keep.tile([128, T], BF16, name=f"attT{m}", tag=f"attT{m}") for m in range(2)]
        ht_r = [keep.tile([128, T], BF16, name=f"ht{k}", tag=f"ht{k}") for k in range(NKD)]
        wq_r = keep.tile([128, NKD * GD], BF16, name="wq_r", tag="wq_r")
        wk_r = keep.tile([128, NKD * GD], BF16, name="wk_r", tag="wk_r")
        wv_r = keep.tile([128, NKD * GV], BF16, name="wv_r", tag="wv_r")
        wo_r = keep.tile([128, 2 * D], BF16, name="wo_r", tag="wo_r")

        # ---- input DMA: sync + gpsimd only (the scalar/ACT queue must stay
        # clean — DMA descriptor preps and ring-full waits on it would stall
        # the exp stream). Strictly need-ordered: weights, then HT token
        # quarters in consumption order.
        nc.sync.dma_start(wq_r[:], WqP[:])
        nc.gpsimd.dma_start(wk_r[:], WkP[:])
        QIO = [nc.sync, nc.gpsimd]
        for q in range(2):
            for k in range(NKD):
                QIO[k % 2].dma_start(
                    ht_r[k][:, q * 512:(q + 1) * 512],
                    HT[k * 128:(k + 1) * 128, q * 512:(q + 1) * 512],
                )
        nc.gpsimd.dma_start(wv_r[:], WvP[:])
        for k in range(NKD):
            QIO[k % 2].dma_start(ht_r[k][:, 1024:2048], HT[k * 128:(k + 1) * 128, 1024:2048])
        nc.gpsimd.dma_start(wo_r[:], WoP[:])

        # lower-tri mask [128,128]: keep where free >= partition
        tri = cpool.tile([128, 128], BF16, name="tri", tag="tri")
        nc.gpsimd.memset(tri[:], 1.0)
        nc.gpsimd.affine_select(
            out=tri[:], in_=tri[:], compare_op=OP.is_ge, fill=0.0,
            base=0, pattern=[[1, 128]], channel_multiplier=-1,
        )

        # ---- emission-time clock model (us) for filler pacing ----
        # PE at 2.4 GHz, ACT at 1.2 GHz, DVE at 0.96 GHz; coarse per-op
        # costs including access penalties. Used only to decide how much
        # projection filler to pack between attention chunks.
        clk = {"pe": 0.0, "act": 0.0, "dve": 0.0}

        def mm_cost(n_cols):
            return n_cols / 2400.0

        def dve_cost(n_free):
            return (n_free + 120) / 960.0

        # ---- units, split into single-matmul steps for fine interleaving ----
        emitted = set()
        ps_of = {}

        def qk_steps(di, m, n):
            # 8 accumulation matmuls + one DVE bias-add copy-out
            key = ("qk", di, m, n)
            if key in emitted:
                return []
            emitted.add(key)
            dest, w_r = ((qT, wq_r), (kT, wk_r))[di]

            def mk_mm(k):
                def f():
                    if k == 0:
                        ps_of[key] = paux.tile(
                            [128, 512], F32, name="ps", tag="aux", bufs=2
                        )
                    nc.tensor.matmul(
                        ps_of[key][:],
                        w_r[:, k * GD + m * 128: k * GD + m * 128 + 128],
                        ht_r[k][:, n * 512:(n + 1) * 512],
                        start=(k == 0), stop=(k == NKD - 1),
                    )
                    clk["pe"] += mm_cost(512)
                return f

            def fin():
                nc.vector.tensor_scalar_add(
                    dest[m][:, n * 512:(n + 1) * 512], ps_of.pop(key)[:],
                    bqk_sb[:, 2 * di + m: 2 * di + m + 1],
                )
                clk["dve"] = max(clk["dve"], clk["pe"]) + dve_cost(512)

            return [mk_mm(k) for k in range(NKD)] + [fin]

        def v_steps(t):
            key = ("v", t)
            if key in emitted:
                return []
            emitted.add(key)

            def mk_mm(k):
                def f():
                    if k == 0:
                        ps_of[key] = paux.tile(
                            [128, 512], F32, name="vp", tag="aux", bufs=2
                        )
                    nc.tensor.matmul(
                        ps_of[key][:, 0:GV],
                        ht_r[k][:, t * 128:(t + 1) * 128],
                        wv_r[:, k * GV:(k + 1) * GV],
                        start=(k == 0), stop=False,
                    )
                    clk["pe"] += mm_cost(GV)
                return f

            def bias_mm():
                nc.tensor.matmul(
                    ps_of[key][:, 0:GV], ones_r[:], bv_r[:], start=False, stop=True
                )
                clk["pe"] += mm_cost(GV)

            def fin():
                nc.vector.tensor_scalar_mul(
                    vt[t][:], ps_of.pop(key)[:, 0:GV], kpm_sb[:, t:t + 1]
                )
                clk["dve"] = max(clk["dve"], clk["pe"]) + dve_cost(GV)

            return [mk_mm(k) for k in range(NKD)] + [bias_mm, fin]

        def c_steps(t):
            ot_box = []
            steps = []
            for n in range(2):
                def mk(n=n):
                    def f():
                        if not ot_box:
                            ot_box.append(
                                work.tile([128, D], BF16, name="ot", tag="ot", bufs=3)
                            )
                        op = paux.tile([128, 512], F32, name="op", tag="aux", bufs=2)
                        for hp in range(2):
                            nc.tensor.matmul(
                                op[:],
                                attT[hp][:, t * 128:(t + 1) * 128],
                                wo_r[:, hp * D + n * 512: hp * D + (n + 1) * 512],
                                start=(hp == 0), stop=(hp == 1),
                            )
                        clk["pe"] += 2 * mm_cost(512)
                        if t >= 12 and n == 1:
                            # tail: exp stream is over, use the idle ACT
                            nc.scalar.copy(ot_box[0][:, n * 512:(n + 1) * 512], op[:])
                        else:
                            nc.vector.tensor_copy(
                                ot_box[0][:, n * 512:(n + 1) * 512], op[:]
                            )
                            clk["dve"] = max(clk["dve"], clk["pe"]) + dve_cost(512)
                    return f
                steps.append(mk())

            def fin():
                nc.sync.dma_start(O[t * 128:(t + 1) * 128, :], ot_box[0][:])

            steps.append(fin)
            return steps

        from collections import deque

        # filler machinery: units are deques of single-op steps; pop_filler
        # advances one step of the front unit; ensure() flushes a unit's
        # remaining steps immediately (hard dependency reached its deadline).
        unit_q = {}
        filler_order = deque()
        open_unit = [None]  # partially-drained filler unit (holds an aux slot)

        def queue_unit(key, steps):
            if steps:
                unit_q[key] = deque(steps)
                filler_order.append(key)

        def _flush(key):
            q = unit_q.pop(key, None)
            if open_unit[0] == key:
                open_unit[0] = None
            while q:
                q.popleft()()

        def ensure(key, steps_fn):
            # The shared-aux PSUM ring has 2 slots: a mid-flight filler unit
            # holds one, an inline ensure takes the other. Flush the open
            # filler first so consecutive ensures can't rotate onto it.
            if open_unit[0] is not None and open_unit[0] != key:
                _flush(open_unit[0])
            if key in unit_q:
                _flush(key)
            else:
                for s in steps_fn():
                    s()

        def pop_filler():
            while filler_order:
                key = filler_order[0]
                q = unit_q.get(key)
                if not q:
                    filler_order.popleft()
                    unit_q.pop(key, None)
                    if open_unit[0] == key:
                        open_unit[0] = None
                    continue
                q.popleft()()
                if not q:
                    filler_order.popleft()
                    unit_q.pop(key, None)
                    if open_unit[0] == key:
                        open_unit[0] = None
                else:
                    open_unit[0] = key
                return True
            return False

        def fillers_left():
            return bool(filler_order)

        def steps_until(keys):
            # steps remaining in the queue prefix that covers all of `keys`
            want = set(k for k in keys if k in unit_q)
            if not want:
                return 0
            tot = 0
            for k in filler_order:
                tot += len(unit_q.get(k, ()))
                want.discard(k)
                if not want:
                    break
            return tot

        # queue order: per J group, deps-first — (J,0)'s scores pair and
        # first diag v, then (J,1)'s scores pair, then the tail diag v's
        queue_unit(("qk", 0, 1, 0), qk_steps(0, 1, 0))
        queue_unit(("qk", 1, 1, 0), qk_steps(1, 1, 0))
        for J in range(1, NJ):
            queue_unit(("qk", 0, 0, J), qk_steps(0, 0, J))
            queue_unit(("qk", 1, 0, J), qk_steps(1, 0, J))
            queue_unit(("v", 4 * J), v_steps(4 * J))
            queue_unit(("qk", 0, 1, J), qk_steps(0, 1, J))
            queue_unit(("qk", 1, 1, J), qk_steps(1, 1, J))
            for t in range(4 * J + 1, 4 * J + 4):
                queue_unit(("v", t), v_steps(t))

        # ---- attention chunk stream ----
        chunks = []
        for J, hp in BLOCKS:
            kcs = [4 * J] + list(range(4 * J)) + [4 * J + i for i in range(1, 4)]
            for ti, kc in enumerate(kcs):
                chunks.append((J, hp, kc, ti, len(kcs)))

        ex_of = {}
        at_of = {}
        at_free = [0.0]  # est time the previous block's at pair frees

        def sc_exp(idx):
            J, hp, kc, ti, _n = chunks[idx]
            if ti == 0:
                ensure(("qk", 0, hp, J), lambda: qk_steps(0, hp, J))
                ensure(("qk", 1, hp, J), lambda: qk_steps(1, hp, J))
            off = max(0, 128 * (kc - 4 * J))
            w = 512 - off
            sc = psc.tile([128, 1024], F32, name="sc", tag="sc", bufs=2)
            for hh in range(2):
                nc.tensor.matmul(
                    sc[:, hh * 512:hh * 512 + w],
                    kT[hp][hh * 64:(hh + 1) * 64, kc * 128:(kc + 1) * 128],
                    qT[hp][hh * 64:(hh + 1) * 64, J * 512 + off:(J + 1) * 512],
                    start=True, stop=True,
                    tile_position=(hh * 64, 0),
                )
            clk["pe"] += mm_cost(w)
            ex = work.tile([128, 1024], BF16, name="ex", tag="ex", bufs=8)
            nc.scalar.activation(
                ex[:].rearrange("p (h c) -> p h c", c=512)[:, :, 0:w],
                sc[:].rearrange("p (h c) -> p h c", c=512)[:, :, 0:w],
                AF.Exp, scale=0.125,
            )
            clk["act"] = max(clk["act"], clk["pe"] + 0.1) + (2 * w + 420) / 1200.0
            ex_ready = clk["act"]
            if off or kc == 4 * J:
                for hh in range(2):
                    nc.vector.tensor_tensor(
                        ex[:, hh * 512:hh * 512 + 128],
                        ex[:, hh * 512:hh * 512 + 128],
                        tri[:],
                        op=OP.mult,
                    )
                clk["dve"] = max(clk["dve"], clk["act"]) + 2 * dve_cost(128)
                ex_ready = clk["dve"]
            ex_of[idx] = (ex, ex_ready)

        def issue_av(idx):
            J, hp, kc, ti, n = chunks[idx]
            ex, ex_ready = ex_of.pop(idx)
            ensure(("v", kc), lambda: v_steps(kc))
            if ti == 0:
                at_of[(J, hp)] = [
                    pav.tile([128, 512], F32, name=f"at{hh}", tag="av", bufs=2)
                    for hh in range(2)
                ]
            at = at_of[(J, hp)]
            off = max(0, 128 * (kc - 4 * J))
            w = 512 - off
            for hh in range(2):
                h = 2 * hp + hh
                nc.tensor.matmul(
                    at[hh][0:65, off:512],
                    vt[kc][:, h * 65:(h + 1) * 65],
                    ex[:, hh * 512:hh * 512 + w],
                    start=(ti == 0), stop=(ti == n - 1),
                )
            dep = max(ex_ready, at_free[0] if ti == 0 else 0.0)
            clk["pe"] = max(clk["pe"], dep) + 2 * mm_cost(w)

        def post_block(J, hp):
            # softmax normalize: one staging cast per head frees the PSUM at
            # banks fast; reciprocal / broadcast / multiply run lazily.
            at = at_of.pop((J, hp))
            stgs = []
            for hh in range(2):
                stg = work.tile([65, 512], F32, name="stg", tag="stg", bufs=4)
                nc.vector.tensor_copy(stg[:], at[hh][0:65, :])
                stgs.append(stg)
            clk["dve"] = max(clk["dve"], clk["pe"]) + 2 * dve_cost(512)
            at_free[0] = clk["dve"]
            for hh in range(2):
                # reciprocal_approx_fast misbehaves on HW when its input AP
                # has a partition offset — hop the denominator row to
                # partition 0 with a plain copy first
                sden = work.tile([1, 512], F32, name="sden", tag="sden", bufs=4)
                nc.vector.tensor_copy(sden[:], stgs[hh][64:65, :])
                rden = work.tile([1, 512], F32, name="rden", tag="rden", bufs=4)
                nc.vector.reciprocal_approx_fast(rden[:], sden[:])
                rb = work.tile([64, 512], F32, name="rb", tag="rb", bufs=4)
                nc.gpsimd.partition_broadcast(rb[:], rden[:])
                nc.vector.tensor_tensor(
                    attT[hp][hh * 64:(hh + 1) * 64, J * 512:(J + 1) * 512],
                    stgs[hh][0:64, :],
                    rb[:],
                    op=OP.mult,
                )
            clk["dve"] += 2 * (3 * dve_cost(512)) + 1.2
            if hp == 1:
                for t in range(4 * J, 4 * J + 4):
                    queue_unit(("c", t), c_steps(t))

        def block_deps(bi):
            # unit keys a block needs before its scores / first AV; later
            # vt are streamed per-chunk by issue_av's ensure
            if bi >= len(BLOCKS):
                return []
            J, hp = BLOCKS[bi]
            return [("qk", 0, hp, J), ("qk", 1, hp, J), ("v", 4 * J)]

        # ---- prologue: minimum before the first exp, then the stream ----
        ensure(("qk", 0, 0, 0), lambda: qk_steps(0, 0, 0))
        ensure(("qk", 1, 0, 0), lambda: qk_steps(1, 0, 0))
        sc_exp(0)
        sc_exp(1)
        bi = 0
        for idx, (J, hp, kc, ti, n) in enumerate(chunks):
            # deadline prefetch (next block's projection deps, spread over
            # this block) + fixed ~0.55us pacing, hard-capped per chunk so
            # the one-chunk exp pipeline buffer never drains (ACT starves
            # after >1 chunk-time of consecutive PE filler).
            rem = steps_until(block_deps(bi + 1))
            left = (n - ti) - 2
            rate = rem if left <= 0 else -(-rem // left)
            pe0 = clk["pe"]
            budget = 1.0 if ti == 0 else 0.45
            popped = 0
            while fillers_left():
                over = clk["pe"] - pe0
                if over >= budget and (popped >= rate or over >= 1.2):
                    break
                pop_filler()
                popped += 1
            if idx + 2 < len(chunks):
                sc_exp(idx + 2)
            issue_av(idx)
            if ti == n - 1:
                post_block(J, hp)
                bi += 1

        # ---- tail: remaining output chunks ----
        while pop_filler():
            pass

    nc.compile()
    _NC_CACHE["nc"] = nc
    return nc


def _prep_core_inputs(H, key_padding_mask, Wq, bq, Wk, bk, Wv, bv, Wo, bo):
    keep = 1.0 - np.asarray(key_padding_mask, dtype=np.float32)  # [B, T]
    bf = ml_dtypes.bfloat16
    in_maps = []
    for c in range(8):
        b, g = divmod(c, 4)
        sl = slice(g * GD, (g + 1) * GD)
        WqT = np.ascontiguousarray(Wq[sl].T)  # [D, GD]
        WkT = np.ascontiguousarray(Wk[sl].T)
        WvT = Wv[sl].T  # [D, GD]
        WvS = np.zeros((D, GV), dtype=np.float32)
        bvS = np.zeros((1, GV), dtype=np.float32)
        for h in range(HPC):
            WvS[:, h * 65:h * 65 + 64] = WvT[:, h * 64:(h + 1) * 64]
            bvS[0, h * 65:h * 65 + 64] = bv[sl][h * 64:(h + 1) * 64]
            bvS[0, h * 65 + 64] = 1.0
        # pack weight k-chunks side by side: [128, NKD*width]
        WqPk = WqT.reshape(NKD, 128, GD).transpose(1, 0, 2).reshape(128, NKD * GD)
        WkPk = WkT.reshape(NKD, 128, GD).transpose(1, 0, 2).reshape(128, NKD * GD)
        WvPk = WvS.reshape(NKD, 128, GV).transpose(1, 0, 2).reshape(128, NKD * GV)
        WoS = np.ascontiguousarray(Wo[:, sl].T)  # [GD, D]
        WoPk = WoS.reshape(2, 128, D).transpose(1, 0, 2).reshape(128, 2 * D)
        bqk_m = np.stack(
            [bq[sl][0:128], bq[sl][128:256], bk[sl][0:128], bk[sl][128:256]], axis=1
        )  # [128, 4]
        in_maps.append({
            "HT": np.ascontiguousarray(H[b].T).astype(bf),
            "WqP": np.ascontiguousarray(WqPk).astype(bf),
            "WkP": np.ascontiguousarray(WkPk).astype(bf),
            "WvP": np.ascontiguousarray(WvPk).astype(bf),
            "WoP": np.ascontiguousarray(WoPk).astype(bf),
            "bqk": np.ascontiguousarray(bqk_m.astype(np.float32)),
            "bvP": bvS.astype(bf),
            "kpm": np.ascontiguousarray(keep[b].reshape(NT, 128).T),
        })
    return in_maps


def kernel(H, key_padding_mask, Wq, bq, Wk, bk, Wv, bv, Wo, bo, _run_kwargs=None):
    H = np.asarray(H, dtype=np.float32)
    Wq = np.asarray(Wq, dtype=np.float32)
    Wk = np.asarray(Wk, dtype=np.float32)
    Wv = np.asarray(Wv, dtype=np.float32)
    Wo = np.asarray(Wo, dtype=np.float32)
    bq = np.asarray(bq, dtype=np.float32)
    bk = np.asarray(bk, dtype=np.float32)
    bv = np.asarray(bv, dtype=np.float32)
    bo = np.asarray(bo, dtype=np.float32)

    nc = build()
    in_maps = _prep_core_inputs(H, key_padding_mask, Wq, bq, Wk, bk, Wv, bv, Wo, bo)
    res = bass_utils.run_bass_kernel_spmd(
        nc, in_maps, core_ids=list(range(8)), **(_run_kwargs or {})
    )
    out = np.zeros((B, T, D), dtype=np.float32)
    for c in range(8):
        out[c // 4] += np.asarray(res.results[c]["O"], dtype=np.float32)
    out += bo
    if _run_kwargs:
        kernel.last_result = res
    return out


# revision 40
# speedup vs baseline: 1.0515x; 1.0136x over previous
"""Multi-head self-attention (B=2, T=2048, D=1024, 16 heads) on 8 TRN2 cores.

Sharding: core c = (b, g) with b = c // 4 (batch), g = c % 4 (head group of 4).
Each core computes q/k/v projections for its 4 heads, causal softmax
attention, and a partial output projection (its 256 columns of the
concat-head dim against Wo). Host sums the 4 partials per batch and adds bo.

The schedule is one flat stream of attention score-chunks (block = (J query
block of 512, hp head pair); chunk = 128 keys) with the scores matmuls
emitted two chunks ahead of their AV consumers, so the scalar engine's Exp
stream — the serial resource of the attention middle — runs back-to-back.
Everything else is demand-driven around that stream:

  - A dozen junk matmuls at t=0 warm the PE's HAM clock gate to 2.4 GHz
    while the first input DMAs (spread over 4 engine queues) land.
  - Projection work (qk per token-quarter, v per key-chunk) is emitted
    lazily: the minimum needed before the first attention block, the rest
    as PE filler popped between chunks / at block starts, with hard
    dependencies ensured at the block that needs them.
  - Softmax: AV accumulates [v|1].T @ exp into at [65,512] PSUM (row 64 =
    denominator via the ones column of v, which also applies the key
    padding mask); at block end one DVE cast stages at into SBUF f32 and
    frees the PSUM bank fast; the reciprocal/broadcast/normalize-multiply
    run lazily off the critical path (DVE + gpsimd), writing bf16 attT.
  - ACT does only Exp (one LUT load) plus the qk bias-add copies
    (Identity + per-partition bias); v masking is a DVE tensor_scalar;
    O-projection PSUM->SBUF casts are DVE; O is stored bf16.
  - PSUM: scores 2x[128,1024] (4 banks) + at 2x[128,512] (2 banks) +
    shared aux tag (2 banks) for warmup/qk/v/O-projection accumulators.

Scores chunks [tk=128, tq<=512] = kT.T @ qT with the head pair row-packed
via tile_position (0,0)/(64,0); diagonal chunks column-restricted to the
causal region and tri-masked on their first 128 columns (DVE multiply).
"""

import ml_dtypes
import numpy as np

import concourse.bass as bass
import concourse.tile as tile
from concourse import bacc, mybir
from concourse import bass_utils
from contextlib import ExitStack

F32 = mybir.dt.float32
BF16 = mybir.dt.bfloat16
AF = mybir.ActivationFunctionType
OP = mybir.AluOpType

B, T, D = 2, 2048, 1024
NH, DH = 16, 64
HPC = 4              # heads per core
GD = HPC * DH        # 256, group dim
GV = HPC * (DH + 1)  # 260, packed v width (64 v dims + ones col per head)
NKD = D // 128       # 8 K-chunks for projections
NT = T // 128        # 16 token chunks
NJ = T // 512        # 4 query blocks

BLOCKS = [(0, 0), (0, 1), (1, 0), (1, 1), (2, 0), (2, 1), (3, 0), (3, 1)]

_NC_CACHE = {}


def build():
    if "nc" in _NC_CACHE:
        return _NC_CACHE["nc"]
    nc = bacc.Bacc("TRN2", target_bir_lowering=False, debug=False, num_devices=8)

    HT = nc.dram_tensor("HT", [D, T], BF16, kind="ExternalInput").ap()
    WqP = nc.dram_tensor("WqP", [128, NKD * GD], BF16, kind="ExternalInput").ap()
    WkP = nc.dram_tensor("WkP", [128, NKD * GD], BF16, kind="ExternalInput").ap()
    WvP = nc.dram_tensor("WvP", [128, NKD * GV], BF16, kind="ExternalInput").ap()
    WoP = nc.dram_tensor("WoP", [128, 2 * D], BF16, kind="ExternalInput").ap()
    bqk = nc.dram_tensor("bqk", [128, 4], F32, kind="ExternalInput").ap()
    bvP = nc.dram_tensor("bvP", [1, GV], BF16, kind="ExternalInput").ap()
    kpm = nc.dram_tensor("kpm", [128, NT], F32, kind="ExternalInput").ap()
    O = nc.dram_tensor("O", [T, D], BF16, kind="ExternalOutput").ap()

    with tile.TileContext(nc) as tc, ExitStack() as octx:
        cpool = octx.enter_context(tc.tile_pool(name="const", bufs=1))
        keep = octx.enter_context(tc.tile_pool(name="keep", bufs=1))
        work = octx.enter_context(tc.tile_pool(name="work", bufs=1))
        psc = octx.enter_context(tc.tile_pool(name="psc", bufs=1, space="PSUM"))
        pav = octx.enter_context(tc.tile_pool(name="pav", bufs=1, space="PSUM"))
        paux = octx.enter_context(tc.tile_pool(name="paux", bufs=1, space="PSUM"))

        # ---- PE warm-up: junk matmuls at t=0 ramp the HAM clock gate while
        # the first input DMAs land. Nothing reads the results.
        junk = cpool.tile([128, 512], BF16, name="junk", tag="junk")
        nc.vector.memset(junk[:], 0.0)
        for _ in range(26):
            wp = paux.tile([128, 512], F32, name="wp", tag="aux", bufs=2)
            nc.tensor.matmul(wp[:, 0:256], junk[:, 0:128], junk[:, 0:256],
                             start=True, stop=True)

        # ---- constants (small DMAs first on the gpsimd queue) ----
        ones_r = cpool.tile([1, 128], BF16, name="ones_r", tag="ones_r")
        nc.vector.memset(ones_r[:], 1.0)
        bqk_sb = cpool.tile([128, 4], F32, name="bqk_sb", tag="bqk_sb")
        nc.gpsimd.dma_start(bqk_sb[:], bqk[:])
        bv_r = cpool.tile([1, GV], BF16, name="bv_r", tag="bv_r")
        nc.gpsimd.dma_start(bv_r[:], bvP[:])
        kpm_sb = cpool.tile([128, NT], F32, name="kpm_sb", tag="kpm_sb")
        nc.gpsimd.dma_start(kpm_sb[:], kpm[:])

        # ---- long-lived tiles ----
        qT = [keep.tile([128, T], BF16, name=f"qT{m}", tag=f"qT{m}") for m in range(2)]
        kT = [keep.tile([128, T], BF16, name=f"kT{m}", tag=f"kT{m}") for m in range(2)]
        vt = [keep.tile([128, GV], BF16, name=f"vt{t}", tag=f"vt{t}") for t in range(NT)]
        attT = [keep.tile([128, T], BF16, name=f"attT{m}", tag=f"attT{m}") for m in range(2)]
        ht_r = [keep.tile([128, T], BF16, name=f"ht{k}", tag=f"ht{k}") for k in range(NKD)]
        wq_r = keep.tile([128, NKD * GD], BF16, name="wq_r", tag="wq_r")
        wk_r = keep.tile([128, NKD * GD], BF16, name="wk_r", tag="wk_r")
        wv_r = keep.tile([128, NKD * GV], BF16, name="wv_r", tag="wv_r")
        wo_r = keep.tile([128, 2 * D], BF16, name="wo_r", tag="wo_r")

        # ---- input DMA: sync + gpsimd only (the scalar/ACT queue must stay
        # clean — DMA descriptor preps and ring-full waits on it would stall
        # the exp stream). Strictly need-ordered: the m=0 weight halves and
        # the first HT token quarter gate the first scores chunk.
        nc.sync.dma_start(wq_r[:, 0:1024], WqP[:, 0:1024])
        nc.gpsimd.dma_start(wk_r[:, 0:1024], WkP[:, 0:1024])
        QIO = [nc.sync, nc.gpsimd]
        for k in range(NKD):
            QIO[k % 2].dma_start(ht_r[k][:, 0:512], HT[k * 128:(k + 1) * 128, 0:512])
        nc.sync.dma_start(wq_r[:, 1024:2048], WqP[:, 1024:2048])
        nc.gpsimd.dma_start(wk_r[:, 1024:2048], WkP[:, 1024:2048])
        for k in range(NKD):
            QIO[k % 2].dma_start(ht_r[k][:, 512:1024], HT[k * 128:(k + 1) * 128, 512:1024])
        nc.gpsimd.dma_start(wv_r[:], WvP[:])
        for k in range(NKD):
            QIO[k % 2].dma_start(ht_r[k][:, 1024:2048], HT[k * 128:(k + 1) * 128, 1024:2048])
        nc.gpsimd.dma_start(wo_r[:], WoP[:])

        # lower-tri mask [128,128]: keep where free >= partition
        tri = cpool.tile([128, 128], BF16, name="tri", tag="tri")
        nc.gpsimd.memset(tri[:], 1.0)
        nc.gpsimd.affine_select(
            out=tri[:], in_=tri[:], compare_op=OP.is_ge, fill=0.0,
            base=0, pattern=[[1, 128]], channel_multiplier=-1,
        )

        # ---- emission-time clock model (us) for filler pacing ----
        # PE at 2.4 GHz, ACT at 1.2 GHz, DVE at 0.96 GHz; coarse per-op
        # costs including access penalties. Used only to decide how much
        # projection filler to pack between attention chunks.
        clk = {"pe": 0.0, "act": 0.0, "dve": 0.0}

        def mm_cost(n_cols):
            return n_cols / 2400.0

        def dve_cost(n_free):
            return (n_free + 120) / 960.0

        # ---- units, split into single-matmul steps for fine interleaving ----
        emitted = set()
        ps_of = {}

        def qk_steps(di, m, n):
            # 8 accumulation matmuls + one DVE bias-add copy-out
            key = ("qk", di, m, n)
            if key in emitted:
                return []
            emitted.add(key)
            dest, w_r = ((qT, wq_r), (kT, wk_r))[di]

            def mk_mm(k):
                def f():
                    if k == 0:
                        ps_of[key] = paux.tile(
                            [128, 512], F32, name="ps", tag="aux", bufs=2
                        )
                    nc.tensor.matmul(
                        ps_of[key][:],
                        w_r[:, m * 1024 + k * 128: m * 1024 + k * 128 + 128],
                        ht_r[k][:, n * 512:(n + 1) * 512],
                        start=(k == 0), stop=(k == NKD - 1),
                    )
                    clk["pe"] += mm_cost(512)
                return f

            def fin():
                nc.vector.tensor_scalar_add(
                    dest[m][:, n * 512:(n + 1) * 512], ps_of.pop(key)[:],
                    bqk_sb[:, 2 * di + m: 2 * di + m + 1],
                )
                clk["dve"] = max(clk["dve"], clk["pe"]) + dve_cost(512)

            return [mk_mm(k) for k in range(NKD)] + [fin]

        def v_steps(t):
            key = ("v", t)
            if key in emitted:
                return []
            emitted.add(key)

            def mk_mm(k):
                def f():
                    if k == 0:
                        ps_of[key] = paux.tile(
                            [128, 512], F32, name="vp", tag="aux", bufs=2
                        )
                    nc.tensor.matmul(
                        ps_of[key][:, 0:GV],
                        ht_r[k][:, t * 128:(t + 1) * 128],
                        wv_r[:, k * GV:(k + 1) * GV],
                        start=(k == 0), stop=False,
                    )
                    clk["pe"] += mm_cost(GV)
                return f

            def bias_mm():
                nc.tensor.matmul(
                    ps_of[key][:, 0:GV], ones_r[:], bv_r[:], start=False, stop=True
                )
                clk["pe"] += mm_cost(GV)

            def fin():
                nc.vector.tensor_scalar_mul(
                    vt[t][:], ps_of.pop(key)[:, 0:GV], kpm_sb[:, t:t + 1]
                )
                clk["dve"] = max(clk["dve"], clk["pe"]) + dve_cost(GV)

            return [mk_mm(k) for k in range(NKD)] + [bias_mm, fin]

        def c_steps(t):
            ot_box = []
            steps = []
            for n in range(2):
                def mk(n=n):
                    def f():
                        if not ot_box:
                            ot_box.append(
                                work.tile([128, D], BF16, name="ot", tag="ot", bufs=3)
                            )
                        op = paux.tile([128, 512], F32, name="op", tag="aux", bufs=2)
                        for hp in range(2):
                            nc.tensor.matmul(
                                op[:],
                                attT[hp][:, t * 128:(t + 1) * 128],
                                wo_r[:, hp * D + n * 512: hp * D + (n + 1) * 512],
                                start=(hp == 0), stop=(hp == 1),
                            )
                        clk["pe"] += 2 * mm_cost(512)
                        if t >= 12 and n == 1:
                            # tail: exp stream is over, use the idle ACT
                            nc.scalar.copy(ot_box[0][:, n * 512:(n + 1) * 512], op[:])
                        else:
                            nc.vector.tensor_copy(
                                ot_box[0][:, n * 512:(n + 1) * 512], op[:]
                            )
                            clk["dve"] = max(clk["dve"], clk["pe"]) + dve_cost(512)
                    return f
                steps.append(mk())

            def fin():
                nc.sync.dma_start(O[t * 128:(t + 1) * 128, :], ot_box[0][:])

            steps.append(fin)
            return steps

        from collections import deque

        # filler machinery: units are deques of single-op steps; pop_filler
        # advances one step of the front unit; ensure() flushes a unit's
        # remaining steps immediately (hard dependency reached its deadline).
        unit_q = {}
        filler_order = deque()
        open_unit = [None]  # partially-drained filler unit (holds an aux slot)

        def queue_unit(key, steps):
            if steps:
                unit_q[key] = deque(steps)
                filler_order.append(key)

        def _flush(key):
            q = unit_q.pop(key, None)
            if open_unit[0] == key:
                open_unit[0] = None
            while q:
                q.popleft()()

        def ensure(key, steps_fn):
            # The shared-aux PSUM ring has 2 slots: a mid-flight filler unit
            # holds one, an inline ensure takes the other. Flush the open
            # filler first so consecutive ensures can't rotate onto it.
            if open_unit[0] is not None and open_unit[0] != key:
                _flush(open_unit[0])
            if key in unit_q:
                _flush(key)
            else:
                for s in steps_fn():
                    s()

        def pop_filler():
            while filler_order:
                key = filler_order[0]
                q = unit_q.get(key)
                if not q:
                    filler_order.popleft()
                    unit_q.pop(key, None)
                    if open_unit[0] == key:
                        open_unit[0] = None
                    continue
                q.popleft()()
                if not q:
                    filler_order.popleft()
                    unit_q.pop(key, None)
                    if open_unit[0] == key:
                        open_unit[0] = None
                else:
                    open_unit[0] = key
                return True
            return False

        def fillers_left():
            return bool(filler_order)

        def steps_until(keys):
            # steps remaining in the queue prefix that covers all of `keys`
            want = set(k for k in keys if k in unit_q)
            if not want:
                return 0
            tot = 0
            for k in filler_order:
                tot += len(unit_q.get(k, ()))
                want.discard(k)
                if not want:
                    break
            return tot

        # queue order: per J group, deps-first — (J,0)'s scores pair and
        # first diag v, then (J,1)'s scores pair, then the tail diag v's
        queue_unit(("qk", 0, 1, 0), qk_steps(0, 1, 0))
        queue_unit(("qk", 1, 1, 0), qk_steps(1, 1, 0))
        for J in range(1, NJ):
            queue_unit(("qk", 0, 0, J), qk_steps(0, 0, J))
            queue_unit(("qk", 1, 0, J), qk_steps(1, 0, J))
            queue_unit(("v", 4 * J), v_steps(4 * J))
            queue_unit(("qk", 0, 1, J), qk_steps(0, 1, J))
            queue_unit(("qk", 1, 1, J), qk_steps(1, 1, J))
            for t in range(4 * J + 1, 4 * J + 4):
                queue_unit(("v", t), v_steps(t))

        # ---- attention chunk stream ----
        chunks = []
        for J, hp in BLOCKS:
            kcs = [4 * J] + list(range(4 * J)) + [4 * J + i for i in range(1, 4)]
            for ti, kc in enumerate(kcs):
                chunks.append((J, hp, kc, ti, len(kcs)))

        ex_of = {}
        at_of = {}
        at_free = [0.0]  # est time the previous block's at pair frees

        def sc_exp(idx):
            J, hp, kc, ti, _n = chunks[idx]
            if ti == 0:
                ensure(("qk", 0, hp, J), lambda: qk_steps(0, hp, J))
                ensure(("qk", 1, hp, J), lambda: qk_steps(1, hp, J))
            off = max(0, 128 * (kc - 4 * J))
            w = 512 - off
            sc = psc.tile([128, 1024], F32, name="sc", tag="sc", bufs=2)
            for hh in range(2):
                nc.tensor.matmul(
                    sc[:, hh * 512:hh * 512 + w],
                    kT[hp][hh * 64:(hh + 1) * 64, kc * 128:(kc + 1) * 128],
                    qT[hp][hh * 64:(hh + 1) * 64, J * 512 + off:(J + 1) * 512],
                    start=True, stop=True,
                    tile_position=(hh * 64, 0),
                )
            clk["pe"] += mm_cost(w)
            ex = work.tile([128, 1024], BF16, name="ex", tag="ex", bufs=8)
            nc.scalar.activation(
                ex[:].rearrange("p (h c) -> p h c", c=512)[:, :, 0:w],
                sc[:].rearrange("p (h c) -> p h c", c=512)[:, :, 0:w],
                AF.Exp, scale=0.125,
            )
            clk["act"] = max(clk["act"], clk["pe"] + 0.1) + (2 * w + 420) / 1200.0
            ex_ready = clk["act"]
            if off or kc == 4 * J:
                for hh in range(2):
                    nc.vector.tensor_tensor(
                        ex[:, hh * 512:hh * 512 + 128],
                        ex[:, hh * 512:hh * 512 + 128],
                        tri[:],
                        op=OP.mult,
                    )
                clk["dve"] = max(clk["dve"], clk["act"]) + 2 * dve_cost(128)
                ex_ready = clk["dve"]
            ex_of[idx] = (ex, ex_ready)

        def issue_av(idx):
            J, hp, kc, ti, n = chunks[idx]
            ex, ex_ready = ex_of.pop(idx)
            ensure(("v", kc), lambda: v_steps(kc))
            if ti == 0:
                at_of[(J, hp)] = [
                    pav.tile([128, 512], F32, name=f"at{hh}", tag="av", bufs=2)
                    for hh in range(2)
                ]
            at = at_of[(J, hp)]
            off = max(0, 128 * (kc - 4 * J))
            w = 512 - off
            for hh in range(2):
                h = 2 * hp + hh
                nc.tensor.matmul(
                    at[hh][0:65, off:512],
                    vt[kc][:, h * 65:(h + 1) * 65],
                    ex[:, hh * 512:hh * 512 + w],
                    start=(ti == 0), stop=(ti == n - 1),
                )
            dep = max(ex_ready, at_free[0] if ti == 0 else 0.0)
            clk["pe"] = max(clk["pe"], dep) + 2 * mm_cost(w)

        def post_block(J, hp):
            # softmax normalize: one staging cast per head frees the PSUM at
            # banks fast; reciprocal / broadcast / multiply run lazily.
            at = at_of.pop((J, hp))
            stgs = []
            for hh in range(2):
                stg = work.tile([65, 512], F32, name="stg", tag="stg", bufs=4)
                nc.vector.tensor_copy(stg[:], at[hh][0:65, :])
                stgs.append(stg)
            clk["dve"] = max(clk["dve"], clk["pe"]) + 2 * dve_cost(512)
            at_free[0] = clk["dve"]
            for hh in range(2):
                # reciprocal_approx_fast misbehaves on HW when its input AP
                # has a partition offset — hop the denominator row to
                # partition 0 with a plain copy first
                sden = work.tile([1, 512], F32, name="sden", tag="sden", bufs=4)
                nc.vector.tensor_copy(sden[:], stgs[hh][64:65, :])
                rden = work.tile([1, 512], F32, name="rden", tag="rden", bufs=4)
                nc.vector.reciprocal_approx_fast(rden[:], sden[:])
                rb = work.tile([64, 512], F32, name="rb", tag="rb", bufs=4)
                nc.gpsimd.partition_broadcast(rb[:], rden[:])
                nc.vector.tensor_tensor(
                    attT[hp][hh * 64:(hh + 1) * 64, J * 512:(J + 1) * 512],
                    stgs[hh][0:64, :],
                    rb[:],
                    op=OP.mult,
                )
            clk["dve"] += 2 * (3 * dve_cost(512)) + 1.2
            if hp == 1:
                for t in range(4 * J, 4 * J + 4):
                    queue_unit(("c", t), c_steps(t))

        def block_deps(bi):
            # unit keys a block needs before its scores / first AV; later
            # vt are streamed per-chunk by issue_av's ensure
            if bi >= len(BLOCKS):
                return []
            J, hp = BLOCKS[bi]
            return [("qk", 0, hp, J), ("qk", 1, hp, J), ("v", 4 * J)]

        # ---- prologue: minimum before the first exp, then the stream ----
        ensure(("qk", 0, 0, 0), lambda: qk_steps(0, 0, 0))
        ensure(("qk", 1, 0, 0), lambda: qk_steps(1, 0, 0))
        sc_exp(0)
        sc_exp(1)
        bi = 0
        for idx, (J, hp, kc, ti, n) in enumerate(chunks):
            # deadline prefetch (next block's projection deps, spread over
            # this block) + fixed ~0.55us pacing, hard-capped per chunk so
            # the one-chunk exp pipeline buffer never drains (ACT starves
            # after >1 chunk-time of consecutive PE filler).
            rem = steps_until(block_deps(bi + 1))
            left = (n - ti) - 2
            rate = rem if left <= 0 else -(-rem // left)
            pe0 = clk["pe"]
            budget = 1.0 if ti == 0 else 0.45
            popped = 0
            while fillers_left():
                over = clk["pe"] - pe0
                if over >= budget and (popped >= rate or over >= 1.2):
                    break
                pop_filler()
                popped += 1
            if idx + 2 < len(chunks):
                sc_exp(idx + 2)
            issue_av(idx)
            if (J, hp) == (3, 1) and ti >= n - 4:
                # final block: query-slice s of at is complete once the
                # off-diagonals and diagonals up to it have accumulated —
                # normalize it and emit its output chunk immediately so the
                # tail pipelines instead of serializing after the last AV
                s = ti - (n - 4)
                at = at_of[(J, hp)]
                for hh in range(2):
                    stg = work.tile([65, 128], F32, name="stgs", tag="stgs", bufs=8)
                    nc.vector.tensor_copy(stg[:], at[hh][0:65, s * 128:(s + 1) * 128])
                    sden = work.tile([1, 128], F32, name="sdens", tag="sdens", bufs=8)
                    nc.vector.tensor_copy(sden[:], stg[64:65, :])
                    rden = work.tile([1, 128], F32, name="rdens", tag="rdens", bufs=8)
                    nc.vector.reciprocal_approx_fast(rden[:], sden[:])
                    rb = work.tile([64, 128], F32, name="rbs", tag="rbs", bufs=8)
                    nc.gpsimd.partition_broadcast(rb[:], rden[:])
                    nc.vector.tensor_tensor(
                        attT[hp][hh * 64:(hh + 1) * 64,
                                 J * 512 + s * 128: J * 512 + (s + 1) * 128],
                        stg[0:64, :],
                        rb[:],
                        op=OP.mult,
                    )
                for st in c_steps(12 + s):
                    st()
            elif ti == n - 1:
                post_block(J, hp)
                bi += 1

        # ---- tail: remaining output chunks ----
        while pop_filler():
            pass

    nc.compile()
    _NC_CACHE["nc"] = nc
    return nc


def _prep_core_inputs(H, key_padding_mask, Wq, bq, Wk, bk, Wv, bv, Wo, bo):
    keep = 1.0 - np.asarray(key_padding_mask, dtype=np.float32)  # [B, T]
    bf = ml_dtypes.bfloat16
    in_maps = []
    for c in range(8):
        b, g = divmod(c, 4)
        sl = slice(g * GD, (g + 1) * GD)
        WqT = np.ascontiguousarray(Wq[sl].T)  # [D, GD]
        WkT = np.ascontiguousarray(Wk[sl].T)
        WvT = Wv[sl].T  # [D, GD]
        WvS = np.zeros((D, GV), dtype=np.float32)
        bvS = np.zeros((1, GV), dtype=np.float32)
        for h in range(HPC):
            WvS[:, h * 65:h * 65 + 64] = WvT[:, h * 64:(h + 1) * 64]
            bvS[0, h * 65:h * 65 + 64] = bv[sl][h * 64:(h + 1) * 64]
            bvS[0, h * 65 + 64] = 1.0
        # q/k weights m-major: [128, m*1024 + k*128 + j] so the m=0 half is
        # one contiguous early DMA; v stays k-chunk-major
        WqPk = WqT.reshape(NKD, 128, 2, 128).transpose(1, 2, 0, 3).reshape(128, NKD * GD)
        WkPk = WkT.reshape(NKD, 128, 2, 128).transpose(1, 2, 0, 3).reshape(128, NKD * GD)
        WvPk = WvS.reshape(NKD, 128, GV).transpose(1, 0, 2).reshape(128, NKD * GV)
        WoS = np.ascontiguousarray(Wo[:, sl].T)  # [GD, D]
        WoPk = WoS.reshape(2, 128, D).transpose(1, 0, 2).reshape(128, 2 * D)
        bqk_m = np.stack(
            [bq[sl][0:128], bq[sl][128:256], bk[sl][0:128], bk[sl][128:256]], axis=1
        )  # [128, 4]
        in_maps.append({
            "HT": np.ascontiguousarray(H[b].T).astype(bf),
            "WqP": np.ascontiguousarray(WqPk).astype(bf),
            "WkP": np.ascontiguousarray(WkPk).astype(bf),
            "WvP": np.ascontiguousarray(WvPk).astype(bf),
            "WoP": np.ascontiguousarray(WoPk).astype(bf),
            "bqk": np.ascontiguousarray(bqk_m.astype(np.float32)),
            "bvP": bvS.astype(bf),
            "kpm": np.ascontiguousarray(keep[b].reshape(NT, 128).T),
        })
    return in_maps


def kernel(H, key_padding_mask, Wq, bq, Wk, bk, Wv, bv, Wo, bo, _run_kwargs=None):
    H = np.asarray(H, dtype=np.float32)
    Wq = np.asarray(Wq, dtype=np.float32)
    Wk = np.asarray(Wk, dtype=np.float32)
    Wv = np.asarray(Wv, dtype=np.float32)
    Wo = np.asarray(Wo, dtype=np.float32)
    bq = np.asarray(bq, dtype=np.float32)
    bk = np.asarray(bk, dtype=np.float32)
    bv = np.asarray(bv, dtype=np.float32)
    bo = np.asarray(bo, dtype=np.float32)

    nc = build()
    in_maps = _prep_core_inputs(H, key_padding_mask, Wq, bq, Wk, bk, Wv, bv, Wo, bo)
    res = bass_utils.run_bass_kernel_spmd(
        nc, in_maps, core_ids=list(range(8)), **(_run_kwargs or {})
    )
    out = np.zeros((B, T, D), dtype=np.float32)
    for c in range(8):
        out[c // 4] += np.asarray(res.results[c]["O"], dtype=np.float32)
    out += bo
    if _run_kwargs:
        kernel.last_result = res
    return out
